# revision 11
# baseline (speedup 1.0000x reference)
"""Sharded multi-head attention for TRN2 (8 NeuronCores).

Problem: B=4, H=16, S=2048, DK=64 attention with boolean mask [B,1,S,S]
(True entries masked out).  The 64 (batch, head) pairs are independent:
core c handles batch c//2, heads (c%2)*8 .. (c%2)*8+8.

v2: two-stream software pipeline.  The scalar engine (exp) is the
bottleneck (~256 x ~1us activations); everything is organized to keep it
100% fed:
  - TWO independent streams (pairs 0-1 and 2-3) run in alternating
    slots.  Each stream owns half of PSUM: sc [128,1024] (2 banks) +
    accA/accB [65,512] (1 bank each).  A stream's qc/pair boundary
    (epilogue, new accumulators) hides under the other stream's steady
    state, so exp never stalls at boundaries.
  - All inputs are DMA'd up front on parallel queues (qkt on sync, vp
    on vector, keep on gpsimd), priority-ordered so slot 0 can start
    within ~2us.
  - Epilogue: acc is copied PSUM->SBUF in one DVE op (frees the PSUM
    bank for the next qc immediately), then recip/broadcast/normalize
    run off the critical path.  Scalar engine does exp ONLY.

Per-slot per-stream: exp over the pair tile [128, 1024] (heads A/B
side by side, scale=1/8 folded, no max-subtraction: scores ~ N(0,1));
QK for the next iteration (two 64-contraction matmuls tile-packed in
PE row groups 0:64 / 64:128, running concurrently); mask multiply on
DVE (bf16 2x, keep_T broadcast over the head dim); one lagged PV pair
(V' = [V | ones] so row 64 accumulates softmax denominators).

All DMAs are partition-major with >=2KB contiguous runs (host
pre-swizzles inputs, ones column baked into V').
"""

import numpy as np
import ml_dtypes
from contextlib import ExitStack

import concourse.bass as bass
import concourse.tile as tile
from concourse import bacc, mybir
from concourse.bass_utils import run_bass_kernel_spmd

B, H, S, DK = 4, 16, 2048, 64
N_CORES = 8
HPC = (B * H) // N_CORES  # heads per core = 8
NPAIR = HPC // 2

P = 128            # k-tile size / partition count
NKT = S // P       # 16 k tiles
QCH = 512          # q chunk per head (pair tile = [128, 1024] = 2 PSUM banks)
NQ = S // QCH      # 4 q chunks

BF16 = mybir.dt.bfloat16
F32 = mybir.dt.float32
BF = ml_dtypes.bfloat16

PV_LAG = 1  # PVs issue one stream-slot late (never head-of-queue stalls)


def build_nc():
    nc = bacc.Bacc(None, target_bir_lowering=False)
    # qkt[pair, 0] = [Q_A^T ; Q_B^T] stacked on partitions, [pair, 1] = K
    qkt_ext = nc.declare_dram_parameter("qkt", [NPAIR, 2, P, S], BF16, isOutput=False)
    # vp[h, p, t, :] = [V[h, t*128+p, :], 1.0]
    vp_ext = nc.declare_dram_parameter("vp", [HPC, P, NKT, DK + 1], BF16, isOutput=False)
    # keep[p, t, q] = not mask[q, t*128+p]
    keep_ext = nc.declare_dram_parameter("keep", [P, NKT, S], BF16, isOutput=False)
    # out_T[h, d, q] (host un-transposes)
    out_ext = nc.declare_dram_parameter("outT", [HPC, DK, S], F32, isOutput=True)

    with tile.TileContext(nc) as tc, ExitStack() as ctx:
        singles = ctx.enter_context(tc.tile_pool(name="singles", bufs=1))
        w_pool = ctx.enter_context(tc.tile_pool(name="wp", bufs=4))
        ep_pool = ctx.enter_context(tc.tile_pool(name="ep", bufs=1))
        ps_pool = ctx.enter_context(tc.tile_pool(name="ps", bufs=1, space="PSUM"))

        # ---- persistent SBUF tiles; all inputs prefetched up front ----
        qT, kT, vpt = {}, {}, {}
        for pr in range(NPAIR):
            qT[pr] = singles.tile([P, S], BF16, name=f"qT{pr}")
            kT[pr] = singles.tile([P, S], BF16, name=f"kT{pr}")
        for h in range(HPC):
            vpt[h] = singles.tile([P, NKT, DK + 1], BF16, name=f"vph{h}")
        keep_sb = singles.tile([P, NKT, S], BF16, name="keep_sb")

        # priority order: slot 0 needs (q0 c0, k0 c0, q2 c0, k2 c0); all kT
        # chunks of pairs 0/2 by ~slot 15; qT chunk c by slot 16*c; pairs
        # 1/3 only from slot 64 on.
        # two DMA rings, interleaved by urgency: keep kt_k is needed at
        # ~slot k (2.2us each); kT chunks of pairs 0/2 by ~slot 15; qT
        # chunk c by slot 16*c; pairs 1/3 from slot 64.
        CH = S // 4
        sync_order = [
            ("q", 0, 0), ("k", 0, 0), ("q", 2, 0), ("k", 2, 0), ("keep", 0),
            ("k", 0, 1), ("k", 2, 1), ("keep", 2),
            ("k", 0, 2), ("k", 2, 2), ("keep", 4),
            ("k", 0, 3), ("k", 2, 3), ("keep", 6),
            ("q", 0, 1), ("q", 2, 1), ("keep", 8),
            ("q", 0, 2), ("q", 2, 2), ("keep", 10),
            ("q", 0, 3), ("q", 2, 3), ("keep", 12),
            ("q", 1, 0), ("k", 1, 0), ("q", 3, 0), ("k", 3, 0), ("keep", 14),
            ("k", 1, 1), ("k", 3, 1), ("k", 1, 2), ("k", 3, 2),
            ("k", 1, 3), ("k", 3, 3), ("q", 1, 1), ("q", 3, 1),
            ("q", 1, 2), ("q", 3, 2), ("q", 1, 3), ("q", 3, 3),
        ]
        for ent in sync_order:
            if ent[0] == "keep":
                kt = ent[1]
                nc.sync.dma_start(out=keep_sb[:, kt], in_=keep_ext[:, kt])
            else:
                qk, pr, c4 = ent[0], ent[1], ent[2]
                cs = slice(c4 * CH, (c4 + 1) * CH)
                dst = qT[pr] if qk == "q" else kT[pr]
                nc.sync.dma_start(
                    out=dst[:, cs], in_=qkt_ext[pr, 0 if qk == "q" else 1, :, cs]
                )
        for h in (0, 1, 4, 5):
            nc.gpsimd.dma_start(out=vpt[h], in_=vp_ext[h])
        for kt in (1, 3):
            nc.gpsimd.dma_start(out=keep_sb[:, kt], in_=keep_ext[:, kt])
        for h in (2, 3, 6, 7):
            nc.gpsimd.dma_start(out=vpt[h], in_=vp_ext[h])
        for kt in (5, 7, 9, 11, 13, 15):
            nc.gpsimd.dma_start(out=keep_sb[:, kt], in_=keep_ext[:, kt])

        # ---- two interleaved streams ----
        streams = []
        for si, prs in enumerate(((0, 1), (2, 3))):
            its = [
                (pr, qc, kt)
                for pr in prs
                for qc in range(NQ)
                for kt in range(NKT)
            ]
            streams.append(
                {"si": si, "iters": its, "sc": None, "accA": None,
                 "accB": None, "pend": [], "dues": []}
            )
        NSLOT = len(streams[0]["iters"])  # 128

        def issue_qk(st, i):
            pr, qc, kt = st["iters"][i]
            si = st["si"]
            q0, k0 = qc * QCH, kt * P
            sc = ps_pool.tile(
                [P, 2 * QCH], F32, tag=f"sc{si}", name=f"sc{si}_{i}", bufs=1
            )
            nc.tensor.matmul(
                sc[:, 0:QCH],
                kT[pr][0:DK, k0 : k0 + P],
                qT[pr][0:DK, q0 : q0 + QCH],
                start=True, stop=True, tile_position=(0, 0),
            )
            nc.tensor.matmul(
                sc[:, QCH : 2 * QCH],
                kT[pr][DK : 2 * DK, k0 : k0 + P],
                qT[pr][DK : 2 * DK, q0 : q0 + QCH],
                start=True, stop=True, tile_position=(64, 0),
            )
            return sc

        def issue_pv(st, ent):
            kt, w, aA, aB, vA, vB = ent
            nc.tensor.matmul(
                aA, vA[:, kt], w[:, 0:QCH],
                start=(kt == 0), stop=(kt == NKT - 1),
            )
            nc.tensor.matmul(
                aB, vB[:, kt], w[:, QCH : 2 * QCH],
                start=(kt == 0), stop=(kt == NKT - 1),
            )

        def ep_copy(st, acc, tg):
            """free the acc PSUM bank fast: sums row via ACT (proven
            partition-64 read), body via one DVE copy"""
            si = st["si"]
            rowF = ep_pool.tile([1, QCH], F32, tag=f"row{si}{tg}", name=f"row{si}{tg}")
            nc.vector.tensor_copy(rowF, acc[DK : DK + 1, :])
            accS = ep_pool.tile([DK, QCH], F32, tag=f"accS{si}{tg}", name=f"accS{si}{tg}")
            nc.vector.tensor_copy(accS, acc[0:DK])
            return rowF, accS

        def ep_norm(st, rowacc, h, qc, tg):
            """recip of sums row, broadcast, normalize, store (off PSUM)"""
            rowF, accS = rowacc
            si = st["si"]
            q0 = qc * QCH
            recipF = ep_pool.tile([1, QCH], F32, tag=f"rF{si}{tg}", name=f"rF{si}{tg}")
            nc.vector.reciprocal_approx_fast(recipF, rowF)
            recipS = ep_pool.tile([1, QCH], BF16, tag=f"rS{si}{tg}", name=f"rS{si}{tg}")
            nc.vector.tensor_copy(recipS, recipF)
            bcS = ep_pool.tile([DK, QCH], BF16, tag=f"bc{si}{tg}", name=f"bc{si}{tg}")
            nc.gpsimd.partition_broadcast(bcS, recipS)
            outf = ep_pool.tile([DK, QCH], F32, tag=f"of{si}{tg}", name=f"of{si}{tg}")
            nc.gpsimd.tensor_mul(outf, accS, bcS)
            nc.gpsimd.dma_start(out=out_ext[h, :, q0 : q0 + QCH], in_=outf)

        # prologue QKs
        for st in streams:
            st["sc"] = issue_qk(st, 0)

        for s in range(NSLOT):
            for st in streams:
                si = st["si"]
                pr, qc, kt = st["iters"][s]
                if kt == 0:
                    st["accA"] = ps_pool.tile(
                        [DK + 1, QCH], F32, tag=f"acc{si}A",
                        name=f"acc{si}A_{pr}_{qc}", bufs=1,
                    )
                    st["accB"] = ps_pool.tile(
                        [DK + 1, QCH], F32, tag=f"acc{si}B",
                        name=f"acc{si}B_{pr}_{qc}", bufs=1,
                    )
                w = w_pool.tile([P, 2 * QCH], BF16, tag=f"w{si}", name=f"w{si}_{s}")
                nc.scalar.activation(
                    w, st["sc"], mybir.ActivationFunctionType.Exp, scale=0.125
                )
                if s + 1 < NSLOT:
                    st["sc"] = issue_qk(st, s + 1)
                # one masked multiply over both heads: keep slice broadcast
                # (stride-0) over the head dim
                q0 = qc * QCH
                keep_slice = keep_sb[:, kt, q0 : q0 + QCH]
                keep2 = bass.AP(
                    tensor=keep_slice.tensor,
                    offset=keep_slice.offset,
                    ap=[keep_slice.ap[0], [0, 2], keep_slice.ap[1]],
                )
                w2 = w.rearrange("p (r q) -> p r q", r=2)
                nc.vector.tensor_mul(w2, w2, keep2)
                hA, hB = 2 * pr, 2 * pr + 1
                st["pend"].append((kt, w, st["accA"], st["accB"], vpt[hA], vpt[hB]))
                if len(st["pend"]) > PV_LAG:
                    ent = st["pend"].pop(0)
                    issue_pv(st, ent)
                    if ent[0] == NKT - 1:
                        # epilogue for the finished (pr, qc): free PSUM now,
                        # normalize/store over the next slots
                        eA, eB = ent[2], ent[3]
                        epr, eqc = st["iters"][s - PV_LAG][0], st["iters"][s - PV_LAG][1]

                        def mk(stc, a, b, h0, q):
                            state = {}

                            def c0():
                                state["sA"] = ep_copy(stc, a, "A")
                                state["sB"] = ep_copy(stc, b, "B")

                            def c1():
                                ep_norm(stc, state["sA"], h0, q, "A")

                            def c2():
                                ep_norm(stc, state["sB"], h0 + 1, q, "B")

                            return c0, c1, c2

                        c0, c1, c2 = mk(st, eA, eB, 2 * epr, eqc)
                        c0()
                        st["dues"] += [(s + 1, c1), (s + 2, c2)]
                while st["dues"] and st["dues"][0][0] <= s:
                    st["dues"].pop(0)[1]()

        # flush
        for st in streams:
            while st["pend"]:
                ent = st["pend"].pop(0)
                issue_pv(st, ent)
                if ent[0] == NKT - 1:
                    eA, eB = ent[2], ent[3]
                    epr, eqc = st["iters"][NSLOT - 1][0], st["iters"][NSLOT - 1][1]
                    sA = ep_copy(st, eA, "A")
                    sB = ep_copy(st, eB, "B")
                    ep_norm(st, sA, 2 * epr, eqc, "A")
                    ep_norm(st, sB, 2 * epr + 1, eqc, "B")
            for _, fn in st["dues"]:
                fn()
            st["dues"] = []
    nc.finalize()
    return nc


_NC_CACHE = {}


def get_nc():
    if "nc" not in _NC_CACHE:
        _NC_CACHE["nc"] = build_nc()
    return _NC_CACHE["nc"]


def kernel(Q, K, V, mask, _trace=False, _tmpdir=None):
    Q = np.asarray(Q, dtype=np.float32)
    K = np.asarray(K, dtype=np.float32)
    V = np.asarray(V, dtype=np.float32)
    mask = np.asarray(mask)

    in_maps = []
    for c in range(N_CORES):
        b, h0 = c // 2, (c % 2) * HPC
        # [pair, {q,k}, 128, S]: partitions 0:64 = head A dims, 64:128 = head B
        qkt = np.empty((NPAIR, 2, P, S), BF)
        qt = Q[b, h0 : h0 + HPC].transpose(0, 2, 1).reshape(NPAIR, 2 * DK, S)
        kt = K[b, h0 : h0 + HPC].transpose(0, 2, 1).reshape(NPAIR, 2 * DK, S)
        qkt[:, 0] = qt
        qkt[:, 1] = kt
        vp = np.empty((HPC, P, NKT, DK + 1), BF)
        vp[:, :, :, 0:DK] = (
            V[b, h0 : h0 + HPC].reshape(HPC, NKT, P, DK).transpose(0, 2, 1, 3)
        )
        vp[:, :, :, DK] = 1.0
        if c % 2 == 0:
            kp = (~mask[b, 0]).T  # [k, q]
            keep = np.ascontiguousarray(
                kp.reshape(NKT, P, S).transpose(1, 0, 2)
            ).astype(BF)
        in_maps.append({"qkt": qkt, "vp": vp, "keep": keep})

    nc = get_nc()
    res = run_bass_kernel_spmd(
        nc, in_maps, core_ids=list(range(N_CORES)), trace=_trace, tmpdir=_tmpdir
    )
    out = np.empty((B, H, S, DK), np.float32)
    for c in range(N_CORES):
        b, h0 = c // 2, (c % 2) * HPC
        out[b, h0 : h0 + HPC] = np.asarray(res.results[c]["outT"]).transpose(0, 2, 1)
    if _trace:
        return out, res
    return out


# revision 14
# speedup vs baseline: 1.5264x; 1.5264x over previous
"""Sharded multi-head attention for TRN2 (8 NeuronCores).

Problem: B=4, H=16, S=2048, DK=64 attention with boolean mask [B,1,S,S]
(True entries masked out).  The 64 (batch, head) pairs are independent:
core c handles batch c//2, heads (c%2)*8 .. (c%2)*8+8.

v2: two-stream software pipeline.  The scalar engine (exp) is the
bottleneck (~256 x ~1us activations); everything is organized to keep it
100% fed:
  - TWO independent streams (pairs 0-1 and 2-3) run in alternating
    slots.  Each stream owns half of PSUM: sc [128,1024] (2 banks) +
    accA/accB [65,512] (1 bank each).  A stream's qc/pair boundary
    (epilogue, new accumulators) hides under the other stream's steady
    state, so exp never stalls at boundaries.
  - All inputs are DMA'd up front on parallel queues (qkt on sync, vp
    on vector, keep on gpsimd), priority-ordered so slot 0 can start
    within ~2us.
  - Epilogue: acc is copied PSUM->SBUF in one DVE op (frees the PSUM
    bank for the next qc immediately), then recip/broadcast/normalize
    run off the critical path.  Scalar engine does exp ONLY.

Per-slot per-stream: exp over the pair tile [128, 1024] (heads A/B
side by side, scale=1/8 folded, no max-subtraction: scores ~ N(0,1));
QK for the next iteration (two 64-contraction matmuls tile-packed in
PE row groups 0:64 / 64:128, running concurrently); mask multiply on
DVE (bf16 2x, keep_T broadcast over the head dim); one lagged PV pair
(V' = [V | ones] so row 64 accumulates softmax denominators).

All DMAs are partition-major with >=2KB contiguous runs (host
pre-swizzles inputs, ones column baked into V').
"""

import numpy as np
import ml_dtypes
from contextlib import ExitStack

import concourse.bass as bass
import concourse.tile as tile
from concourse import bacc, mybir
from concourse.bass_utils import run_bass_kernel_spmd

B, H, S, DK = 4, 16, 2048, 64
N_CORES = 8
HPC = (B * H) // N_CORES  # heads per core = 8
NPAIR = HPC // 2

P = 128            # k-tile size / partition count
NKT = S // P       # 16 k tiles
QCH = 512          # q chunk per head (pair tile = [128, 1024] = 2 PSUM banks)
NQ = S // QCH      # 4 q chunks

BF16 = mybir.dt.bfloat16
F32 = mybir.dt.float32
BF = ml_dtypes.bfloat16

PV_LAG = 1  # PVs issue one stream-slot late (never head-of-queue stalls)


def build_nc():
    nc = bacc.Bacc(None, target_bir_lowering=False)
    # qkt[pair, 0] = [Q_A^T ; Q_B^T] stacked on partitions, [pair, 1] = K
    qkt_ext = nc.declare_dram_parameter("qkt", [NPAIR, 2, P, S], BF16, isOutput=False)
    # vp[h, p, t, :] = [V[h, t*128+p, :], 1.0]
    vp_ext = nc.declare_dram_parameter("vp", [HPC, P, NKT, DK + 1], BF16, isOutput=False)
    # keep[p, t, q] = not mask[q, t*128+p]
    keep_ext = nc.declare_dram_parameter("keep", [P, NKT, S], BF16, isOutput=False)
    # out_T[h, d, q] (host un-transposes)
    out_ext = nc.declare_dram_parameter("outT", [HPC, DK, S], F32, isOutput=True)

    with tile.TileContext(nc) as tc, ExitStack() as ctx:
        singles = ctx.enter_context(tc.tile_pool(name="singles", bufs=1))
        w_pool = ctx.enter_context(tc.tile_pool(name="wp", bufs=4))
        ep_pool = ctx.enter_context(tc.tile_pool(name="ep", bufs=1))
        ps_pool = ctx.enter_context(tc.tile_pool(name="ps", bufs=1, space="PSUM"))

        # ---- persistent SBUF tiles; all inputs prefetched up front ----
        qT, kT, vpt = {}, {}, {}
        for pr in range(NPAIR):
            qT[pr] = singles.tile([P, S], BF16, name=f"qT{pr}")
            kT[pr] = singles.tile([P, S], BF16, name=f"kT{pr}")
        for h in range(HPC):
            vpt[h] = singles.tile([P, NKT, DK + 1], BF16, name=f"vph{h}")
        keep_sb = singles.tile([P, NKT, S], BF16, name="keep_sb")

        # priority order: slot 0 needs (q0 c0, k0 c0, q2 c0, k2 c0); all kT
        # chunks of pairs 0/2 by ~slot 15; qT chunk c by slot 16*c; pairs
        # 1/3 only from slot 64 on.
        # two DMA rings, interleaved by urgency: keep kt_k is needed at
        # ~slot k (2.2us each); kT chunks of pairs 0/2 by ~slot 15; qT
        # chunk c by slot 16*c; pairs 1/3 from slot 64.
        CH = S // 4
        sync_order = [
            ("q", 0, 0), ("k", 0, 0), ("q", 2, 0), ("k", 2, 0), ("keep", 0),
            ("k", 0, 1), ("k", 2, 1), ("keep", 2),
            ("k", 0, 2), ("k", 2, 2), ("keep", 4),
            ("k", 0, 3), ("k", 2, 3), ("keep", 6),
            ("q", 0, 1), ("q", 2, 1), ("keep", 8),
            ("q", 0, 2), ("q", 2, 2), ("keep", 10),
            ("q", 0, 3), ("q", 2, 3), ("keep", 12),
            ("q", 1, 0), ("k", 1, 0), ("q", 3, 0), ("k", 3, 0), ("keep", 14),
            ("k", 1, 1), ("k", 3, 1), ("k", 1, 2), ("k", 3, 2),
            ("k", 1, 3), ("k", 3, 3), ("q", 1, 1), ("q", 3, 1),
            ("q", 1, 2), ("q", 3, 2), ("q", 1, 3), ("q", 3, 3),
        ]
        for ent in sync_order:
            if ent[0] == "keep":
                kt = ent[1]
                nc.sync.dma_start(out=keep_sb[:, kt], in_=keep_ext[:, kt])
            else:
                qk, pr, c4 = ent[0], ent[1], ent[2]
                cs = slice(c4 * CH, (c4 + 1) * CH)
                dst = qT[pr] if qk == "q" else kT[pr]
                nc.sync.dma_start(
                    out=dst[:, cs], in_=qkt_ext[pr, 0 if qk == "q" else 1, :, cs]
                )
        for h in (0, 1, 4, 5):
            nc.gpsimd.dma_start(out=vpt[h], in_=vp_ext[h])
        for kt in (1, 3):
            nc.gpsimd.dma_start(out=keep_sb[:, kt], in_=keep_ext[:, kt])
        for h in (2, 3, 6, 7):
            nc.gpsimd.dma_start(out=vpt[h], in_=vp_ext[h])
        for kt in (5, 7, 9, 11, 13, 15):
            nc.gpsimd.dma_start(out=keep_sb[:, kt], in_=keep_ext[:, kt])

        # ---- two interleaved streams ----
        streams = []
        for si, prs in enumerate(((0, 1), (2, 3))):
            its = [
                (pr, qc, kt)
                for pr in prs
                for qc in range(NQ)
                for kt in range(NKT)
            ]
            streams.append(
                {"si": si, "iters": its, "sc": None, "accA": None,
                 "accB": None, "pend": [], "dues": []}
            )
        NSLOT = len(streams[0]["iters"])  # 128

        def issue_qk(st, i):
            pr, qc, kt = st["iters"][i]
            si = st["si"]
            q0, k0 = qc * QCH, kt * P
            sc = ps_pool.tile(
                [P, 2 * QCH], F32, tag=f"sc{si}", name=f"sc{si}_{i}", bufs=1
            )
            nc.tensor.matmul(
                sc[:, 0:QCH],
                kT[pr][0:DK, k0 : k0 + P],
                qT[pr][0:DK, q0 : q0 + QCH],
                start=True, stop=True, tile_position=(0, 0),
            )
            nc.tensor.matmul(
                sc[:, QCH : 2 * QCH],
                kT[pr][DK : 2 * DK, k0 : k0 + P],
                qT[pr][DK : 2 * DK, q0 : q0 + QCH],
                start=True, stop=True, tile_position=(64, 0),
            )
            return sc

        def issue_pv(st, ent):
            kt, _, _, w, aA, aB, vA, vB = ent
            nc.tensor.matmul(
                aA, vA[:, kt], w[:, 0:QCH],
                start=(kt == 0), stop=(kt == NKT - 1),
            )
            nc.tensor.matmul(
                aB, vB[:, kt], w[:, QCH : 2 * QCH],
                start=(kt == 0), stop=(kt == NKT - 1),
            )

        def ep_copy(st, acc, tg):
            """free the acc PSUM bank fast: sums row via ACT (proven
            partition-64 read), body via one DVE copy"""
            si = st["si"]
            rowF = ep_pool.tile([1, QCH], F32, tag=f"row{si}{tg}", name=f"row{si}{tg}")
            nc.vector.tensor_copy(rowF, acc[DK : DK + 1, :])
            accS = ep_pool.tile([DK, QCH], F32, tag=f"accS{si}{tg}", name=f"accS{si}{tg}")
            nc.vector.tensor_copy(accS, acc[0:DK])
            return rowF, accS

        def ep_norm(st, rowacc, h, qc, tg):
            """recip of sums row, broadcast, normalize, store (off PSUM)"""
            rowF, accS = rowacc
            si = st["si"]
            q0 = qc * QCH
            recipF = ep_pool.tile([1, QCH], F32, tag=f"rF{si}{tg}", name=f"rF{si}{tg}")
            nc.vector.reciprocal_approx_fast(recipF, rowF)
            recipS = ep_pool.tile([1, QCH], BF16, tag=f"rS{si}{tg}", name=f"rS{si}{tg}")
            nc.gpsimd.tensor_copy(recipS, recipF)
            bcS = ep_pool.tile([DK, QCH], BF16, tag=f"bc{si}{tg}", name=f"bc{si}{tg}")
            nc.gpsimd.partition_broadcast(bcS, recipS)
            outf = ep_pool.tile([DK, QCH], F32, tag=f"of{si}{tg}", name=f"of{si}{tg}")
            nc.vector.tensor_mul(outf, accS, bcS)
            nc.gpsimd.dma_start(out=out_ext[h, :, q0 : q0 + QCH], in_=outf)

        # prologue QKs
        for st in streams:
            st["sc"] = issue_qk(st, 0)

        for s in range(NSLOT):
            for st in streams:
                si = st["si"]
                pr, qc, kt = st["iters"][s]
                if kt == 0:
                    st["accA"] = ps_pool.tile(
                        [DK + 1, QCH], F32, tag=f"acc{si}A",
                        name=f"acc{si}A_{pr}_{qc}", bufs=1,
                    )
                    st["accB"] = ps_pool.tile(
                        [DK + 1, QCH], F32, tag=f"acc{si}B",
                        name=f"acc{si}B_{pr}_{qc}", bufs=1,
                    )
                w = w_pool.tile([P, 2 * QCH], BF16, tag=f"w{si}", name=f"w{si}_{s}")
                nc.scalar.activation(
                    w, st["sc"], mybir.ActivationFunctionType.Exp, scale=0.125
                )
                if s + 1 < NSLOT:
                    st["sc"] = issue_qk(st, s + 1)
                # lagged PV + epilogue PSUM-freeing copies go on the engine
                # queues BEFORE this slot's mask multiply, so a new qc's
                # first PV is never gated on DVE work queued behind the mul
                if len(st["pend"]) > PV_LAG:
                    ent = st["pend"].pop(0)
                    issue_pv(st, ent)
                    if ent[0] == NKT - 1:
                        # epilogue for the finished (pr, qc): free PSUM now,
                        # normalize/store over the next slots
                        _, epr, eqc, _, eA, eB, _, _ = ent

                        def mk(stc, a, b, h0, q):
                            state = {}

                            def c0():
                                state["sA"] = ep_copy(stc, a, "A")
                                state["sB"] = ep_copy(stc, b, "B")

                            def c1():
                                ep_norm(stc, state["sA"], h0, q, "A")

                            def c2():
                                ep_norm(stc, state["sB"], h0 + 1, q, "B")

                            return c0, c1, c2

                        c0, c1, c2 = mk(st, eA, eB, 2 * epr, eqc)
                        c0()
                        st["dues"] += [(s + 1, c1), (s + 2, c2)]
                while st["dues"] and st["dues"][0][0] <= s:
                    st["dues"].pop(0)[1]()
                # one masked multiply over both heads: keep slice broadcast
                # (stride-0) over the head dim
                q0 = qc * QCH
                keep_slice = keep_sb[:, kt, q0 : q0 + QCH]
                keep2 = bass.AP(
                    tensor=keep_slice.tensor,
                    offset=keep_slice.offset,
                    ap=[keep_slice.ap[0], [0, 2], keep_slice.ap[1]],
                )
                w2 = w.rearrange("p (r q) -> p r q", r=2)
                nc.vector.tensor_mul(w2, w2, keep2)
                hA, hB = 2 * pr, 2 * pr + 1
                st["pend"].append(
                    (kt, pr, qc, w, st["accA"], st["accB"], vpt[hA], vpt[hB])
                )

        # flush
        for st in streams:
            while st["pend"]:
                ent = st["pend"].pop(0)
                issue_pv(st, ent)
                if ent[0] == NKT - 1:
                    _, epr, eqc, _, eA, eB, _, _ = ent
                    sA = ep_copy(st, eA, "A")
                    sB = ep_copy(st, eB, "B")
                    ep_norm(st, sA, 2 * epr, eqc, "A")
                    ep_norm(st, sB, 2 * epr + 1, eqc, "B")
            for _, fn in st["dues"]:
                fn()
            st["dues"] = []
    nc.finalize()
    return nc


_NC_CACHE = {}


def get_nc():
    if "nc" not in _NC_CACHE:
        _NC_CACHE["nc"] = build_nc()
    return _NC_CACHE["nc"]


def kernel(Q, K, V, mask, _trace=False, _tmpdir=None):
    Q = np.asarray(Q, dtype=np.float32)
    K = np.asarray(K, dtype=np.float32)
    V = np.asarray(V, dtype=np.float32)
    mask = np.asarray(mask)

    in_maps = []
    for c in range(N_CORES):
        b, h0 = c // 2, (c % 2) * HPC
        # [pair, {q,k}, 128, S]: partitions 0:64 = head A dims, 64:128 = head B
        qkt = np.empty((NPAIR, 2, P, S), BF)
        qt = Q[b, h0 : h0 + HPC].transpose(0, 2, 1).reshape(NPAIR, 2 * DK, S)
        kt = K[b, h0 : h0 + HPC].transpose(0, 2, 1).reshape(NPAIR, 2 * DK, S)
        qkt[:, 0] = qt
        qkt[:, 1] = kt
        vp = np.empty((HPC, P, NKT, DK + 1), BF)
        vp[:, :, :, 0:DK] = (
            V[b, h0 : h0 + HPC].reshape(HPC, NKT, P, DK).transpose(0, 2, 1, 3)
        )
        vp[:, :, :, DK] = 1.0
        if c % 2 == 0:
            kp = (~mask[b, 0]).T  # [k, q]
            keep = np.ascontiguousarray(
                kp.reshape(NKT, P, S).transpose(1, 0, 2)
            ).astype(BF)
        in_maps.append({"qkt": qkt, "vp": vp, "keep": keep})

    nc = get_nc()
    res = run_bass_kernel_spmd(
        nc, in_maps, core_ids=list(range(N_CORES)), trace=_trace, tmpdir=_tmpdir
    )
    out = np.empty((B, H, S, DK), np.float32)
    for c in range(N_CORES):
        b, h0 = c // 2, (c % 2) * HPC
        out[b, h0 : h0 + HPC] = np.asarray(res.results[c]["outT"]).transpose(0, 2, 1)
    if _trace:
        return out, res
    return out


# revision 16
# speedup vs baseline: 1.7108x; 1.1208x over previous
"""Sharded multi-head attention for TRN2 (8 NeuronCores).

Problem: B=4, H=16, S=2048, DK=64 attention with boolean mask [B,1,S,S]
(True entries masked out).  The 64 (batch, head) pairs are independent:
core c handles batch c//2, heads (c%2)*8 .. (c%2)*8+8.

v2: two-stream software pipeline.  The scalar engine (exp) is the
bottleneck (~256 x ~1us activations); everything is organized to keep it
100% fed:
  - TWO independent streams (pairs 0-1 and 2-3) run in alternating
    slots.  Each stream owns half of PSUM: sc [128,1024] (2 banks) +
    accA/accB [65,512] (1 bank each).  A stream's qc/pair boundary
    (epilogue, new accumulators) hides under the other stream's steady
    state, so exp never stalls at boundaries.
  - All inputs are DMA'd up front on parallel queues (qkt on sync, vp
    on vector, keep on gpsimd), priority-ordered so slot 0 can start
    within ~2us.
  - Epilogue: acc is copied PSUM->SBUF in one DVE op (frees the PSUM
    bank for the next qc immediately), then recip/broadcast/normalize
    run off the critical path.  Scalar engine does exp ONLY.

Per-slot per-stream: exp over the pair tile [128, 1024] (heads A/B
side by side, scale=1/8 folded, no max-subtraction: scores ~ N(0,1));
QK for the next iteration (two 64-contraction matmuls tile-packed in
PE row groups 0:64 / 64:128, running concurrently); mask multiply on
DVE (bf16 2x, keep_T broadcast over the head dim); one lagged PV pair
(V' = [V | ones] so row 64 accumulates softmax denominators).

All DMAs are partition-major with >=2KB contiguous runs (host
pre-swizzles inputs, ones column baked into V').
"""

import numpy as np
import ml_dtypes
from contextlib import ExitStack

import concourse.bass as bass
import concourse.tile as tile
from concourse import bacc, mybir
from concourse.bass_utils import run_bass_kernel_spmd

B, H, S, DK = 4, 16, 2048, 64
N_CORES = 8
HPC = (B * H) // N_CORES  # heads per core = 8
NPAIR = HPC // 2

P = 128            # k-tile size / partition count
NKT = S // P       # 16 k tiles
QCH = 512          # q chunk per head (pair tile = [128, 1024] = 2 PSUM banks)
NQ = S // QCH      # 4 q chunks

BF16 = mybir.dt.bfloat16
F32 = mybir.dt.float32
BF = ml_dtypes.bfloat16

PV_LAG = 1  # PVs issue one stream-slot late (never head-of-queue stalls)


def build_nc():
    nc = bacc.Bacc(None, target_bir_lowering=False)
    # qkt[pair, 0] = [Q_A^T ; Q_B^T] stacked on partitions, [pair, 1] = K
    qkt_ext = nc.declare_dram_parameter("qkt", [NPAIR, 2, P, S], BF16, isOutput=False)
    # vp[h, p, t, :] = [V[h, t*128+p, :], 1.0]
    vp_ext = nc.declare_dram_parameter("vp", [HPC, P, NKT, DK + 1], BF16, isOutput=False)
    # keep[p, t, q] = not mask[q, t*128+p]
    keep_ext = nc.declare_dram_parameter("keep", [P, NKT, S], BF16, isOutput=False)
    # out_T[h, d, q] (host un-transposes)
    out_ext = nc.declare_dram_parameter("outT", [HPC, DK, S], F32, isOutput=True)

    with tile.TileContext(nc) as tc, ExitStack() as ctx:
        singles = ctx.enter_context(tc.tile_pool(name="singles", bufs=1))
        w_pool = ctx.enter_context(tc.tile_pool(name="wp", bufs=4))
        ep_pool = ctx.enter_context(tc.tile_pool(name="ep", bufs=1))
        ps_pool = ctx.enter_context(tc.tile_pool(name="ps", bufs=1, space="PSUM"))

        # ---- persistent SBUF tiles; all inputs prefetched up front ----
        qT, kT, vpt = {}, {}, {}
        for pr in range(NPAIR):
            qT[pr] = singles.tile([P, S], BF16, name=f"qT{pr}")
            kT[pr] = singles.tile([P, S], BF16, name=f"kT{pr}")
        for h in range(HPC):
            vpt[h] = singles.tile([P, NKT, DK + 1], BF16, name=f"vph{h}")
        keep_sb = singles.tile([P, NKT, S], BF16, name="keep_sb")

        # priority order: slot 0 needs (q0 c0, k0 c0, q2 c0, k2 c0); all kT
        # chunks of pairs 0/2 by ~slot 15; qT chunk c by slot 16*c; pairs
        # 1/3 only from slot 64 on.
        # two DMA rings, interleaved by urgency: keep kt_k is needed at
        # ~slot k (2.2us each); kT chunks of pairs 0/2 by ~slot 15; qT
        # chunk c by slot 16*c; pairs 1/3 from slot 64.
        CH = S // 4
        qk_order = [
            (0, 0, 0), (0, 1, 0), (2, 0, 0), (2, 1, 0),
            (0, 1, 1), (2, 1, 1), (0, 1, 2), (2, 1, 2), (0, 1, 3), (2, 1, 3),
            (0, 0, 1), (2, 0, 1), (0, 0, 2), (2, 0, 2), (0, 0, 3), (2, 0, 3),
            (1, 0, 0), (1, 1, 0), (3, 0, 0), (3, 1, 0),
            (1, 1, 1), (3, 1, 1), (1, 1, 2), (3, 1, 2), (1, 1, 3), (3, 1, 3),
            (1, 0, 1), (3, 0, 1), (1, 0, 2), (3, 0, 2), (1, 0, 3), (3, 0, 3),
        ]
        for pr, qk, c4 in qk_order:
            cs = slice(c4 * CH, (c4 + 1) * CH)
            dst = qT[pr] if qk == 0 else kT[pr]
            nc.sync.dma_start(out=dst[:, cs], in_=qkt_ext[pr, qk, :, cs])
        for h in (0, 1, 4, 5):
            nc.gpsimd.dma_start(out=vpt[h], in_=vp_ext[h])
        for kt in range(8):
            nc.gpsimd.dma_start(out=keep_sb[:, kt], in_=keep_ext[:, kt])
        for h in (2, 3, 6, 7):
            nc.gpsimd.dma_start(out=vpt[h], in_=vp_ext[h])
        for kt in range(8, NKT):
            nc.gpsimd.dma_start(out=keep_sb[:, kt], in_=keep_ext[:, kt])

        # ---- two interleaved streams ----
        streams = []
        for si, prs in enumerate(((0, 1), (2, 3))):
            its = [
                (pr, qc, kt)
                for pr in prs
                for qc in range(NQ)
                for kt in range(NKT)
            ]
            streams.append(
                {"si": si, "iters": its, "sc": None, "accA": None,
                 "accB": None, "pend": [], "dues": []}
            )
        NSLOT = len(streams[0]["iters"])  # 128

        def issue_qk(st, i):
            pr, qc, kt = st["iters"][i]
            si = st["si"]
            q0, k0 = qc * QCH, kt * P
            sc = ps_pool.tile(
                [P, 2 * QCH], F32, tag=f"sc{si}", name=f"sc{si}_{i}", bufs=1
            )
            nc.tensor.matmul(
                sc[:, 0:QCH],
                kT[pr][0:DK, k0 : k0 + P],
                qT[pr][0:DK, q0 : q0 + QCH],
                start=True, stop=True, tile_position=(0, 0),
            )
            nc.tensor.matmul(
                sc[:, QCH : 2 * QCH],
                kT[pr][DK : 2 * DK, k0 : k0 + P],
                qT[pr][DK : 2 * DK, q0 : q0 + QCH],
                start=True, stop=True, tile_position=(64, 0),
            )
            return sc

        def issue_pv(st, ent):
            kt, _, _, w, aA, aB, vA, vB = ent
            nc.tensor.matmul(
                aA, vA[:, kt], w[:, 0:QCH],
                start=(kt == 0), stop=(kt == NKT - 1),
            )
            nc.tensor.matmul(
                aB, vB[:, kt], w[:, QCH : 2 * QCH],
                start=(kt == 0), stop=(kt == NKT - 1),
            )

        def ep_copy(st, acc, tg):
            """free the acc PSUM bank fast: sums row via ACT (proven
            partition-64 read), body via one DVE copy"""
            si = st["si"]
            rowF = ep_pool.tile([1, QCH], F32, tag=f"row{si}{tg}", name=f"row{si}{tg}")
            nc.vector.tensor_copy(rowF, acc[DK : DK + 1, :])
            accS = ep_pool.tile([DK, QCH], F32, tag=f"accS{si}{tg}", name=f"accS{si}{tg}")
            nc.vector.tensor_copy(accS, acc[0:DK])
            return rowF, accS

        def ep_norm(st, rowacc, h, qc, tg):
            """recip of sums row, broadcast, normalize, store (off PSUM)"""
            rowF, accS = rowacc
            si = st["si"]
            q0 = qc * QCH
            recipF = ep_pool.tile([1, QCH], F32, tag=f"rF{si}{tg}", name=f"rF{si}{tg}")
            nc.vector.reciprocal_approx_fast(recipF, rowF)
            recipS = ep_pool.tile([1, QCH], BF16, tag=f"rS{si}{tg}", name=f"rS{si}{tg}")
            nc.vector.tensor_copy(recipS, recipF)
            bcS = ep_pool.tile([DK, QCH], BF16, tag=f"bc{si}{tg}", name=f"bc{si}{tg}")
            nc.gpsimd.partition_broadcast(bcS, recipS)
            outf = ep_pool.tile([DK, QCH], F32, tag=f"of{si}{tg}", name=f"of{si}{tg}")
            nc.vector.tensor_mul(outf, accS, bcS)
            nc.gpsimd.dma_start(out=out_ext[h, :, q0 : q0 + QCH], in_=outf)

        # prologue QKs
        for st in streams:
            st["sc"] = issue_qk(st, 0)

        for s in range(NSLOT):
            for st in streams:
                si = st["si"]
                pr, qc, kt = st["iters"][s]
                if kt == 0:
                    st["accA"] = ps_pool.tile(
                        [DK + 1, QCH], F32, tag=f"acc{si}A",
                        name=f"acc{si}A_{pr}_{qc}", bufs=1,
                    )
                    st["accB"] = ps_pool.tile(
                        [DK + 1, QCH], F32, tag=f"acc{si}B",
                        name=f"acc{si}B_{pr}_{qc}", bufs=1,
                    )
                w = w_pool.tile([P, 2 * QCH], BF16, tag=f"w{si}", name=f"w{si}_{s}")
                nc.scalar.activation(
                    w, st["sc"], mybir.ActivationFunctionType.Exp, scale=0.125
                )
                if s + 1 < NSLOT:
                    st["sc"] = issue_qk(st, s + 1)
                # one masked multiply over both heads: keep slice broadcast
                # (stride-0) over the head dim
                q0 = qc * QCH
                keep_slice = keep_sb[:, kt, q0 : q0 + QCH]
                keep2 = bass.AP(
                    tensor=keep_slice.tensor,
                    offset=keep_slice.offset,
                    ap=[keep_slice.ap[0], [0, 2], keep_slice.ap[1]],
                )
                w2 = w.rearrange("p (r q) -> p r q", r=2)
                nc.vector.tensor_mul(w2, w2, keep2)
                hA, hB = 2 * pr, 2 * pr + 1
                st["pend"].append(
                    (kt, pr, qc, w, st["accA"], st["accB"], vpt[hA], vpt[hB])
                )
                if len(st["pend"]) > PV_LAG:
                    ent = st["pend"].pop(0)
                    issue_pv(st, ent)
                    if ent[0] == NKT - 1:
                        # epilogue for the finished (pr, qc): free PSUM now,
                        # normalize/store over the next slots
                        _, epr, eqc, _, eA, eB, _, _ = ent

                        def mk(stc, a, b, h0, q):
                            state = {}

                            def c0():
                                state["sA"] = ep_copy(stc, a, "A")
                                state["sB"] = ep_copy(stc, b, "B")

                            def c1():
                                ep_norm(stc, state["sA"], h0, q, "A")

                            def c2():
                                ep_norm(stc, state["sB"], h0 + 1, q, "B")

                            return c0, c1, c2

                        c0, c1, c2 = mk(st, eA, eB, 2 * epr, eqc)
                        c0()
                        st["dues"] += [(s + 1, c1), (s + 2, c2)]
                while st["dues"] and st["dues"][0][0] <= s:
                    st["dues"].pop(0)[1]()

        # flush
        for st in streams:
            while st["pend"]:
                ent = st["pend"].pop(0)
                issue_pv(st, ent)
                if ent[0] == NKT - 1:
                    _, epr, eqc, _, eA, eB, _, _ = ent
                    sA = ep_copy(st, eA, "A")
                    sB = ep_copy(st, eB, "B")
                    ep_norm(st, sA, 2 * epr, eqc, "A")
                    ep_norm(st, sB, 2 * epr + 1, eqc, "B")
            for _, fn in st["dues"]:
                fn()
            st["dues"] = []
    nc.finalize()
    return nc


_NC_CACHE = {}


def get_nc():
    if "nc" not in _NC_CACHE:
        _NC_CACHE["nc"] = build_nc()
    return _NC_CACHE["nc"]


def kernel(Q, K, V, mask, _trace=False, _tmpdir=None):
    Q = np.asarray(Q, dtype=np.float32)
    K = np.asarray(K, dtype=np.float32)
    V = np.asarray(V, dtype=np.float32)
    mask = np.asarray(mask)

    in_maps = []
    for c in range(N_CORES):
        b, h0 = c // 2, (c % 2) * HPC
        # [pair, {q,k}, 128, S]: partitions 0:64 = head A dims, 64:128 = head B
        qkt = np.empty((NPAIR, 2, P, S), BF)
        qt = Q[b, h0 : h0 + HPC].transpose(0, 2, 1).reshape(NPAIR, 2 * DK, S)
        kt = K[b, h0 : h0 + HPC].transpose(0, 2, 1).reshape(NPAIR, 2 * DK, S)
        qkt[:, 0] = qt
        qkt[:, 1] = kt
        vp = np.empty((HPC, P, NKT, DK + 1), BF)
        vp[:, :, :, 0:DK] = (
            V[b, h0 : h0 + HPC].reshape(HPC, NKT, P, DK).transpose(0, 2, 1, 3)
        )
        vp[:, :, :, DK] = 1.0
        if c % 2 == 0:
            kp = (~mask[b, 0]).T  # [k, q]
            keep = np.ascontiguousarray(
                kp.reshape(NKT, P, S).transpose(1, 0, 2)
            ).astype(BF)
        in_maps.append({"qkt": qkt, "vp": vp, "keep": keep})

    nc = get_nc()
    res = run_bass_kernel_spmd(
        nc, in_maps, core_ids=list(range(N_CORES)), trace=_trace, tmpdir=_tmpdir
    )
    out = np.empty((B, H, S, DK), np.float32)
    for c in range(N_CORES):
        b, h0 = c // 2, (c % 2) * HPC
        out[b, h0 : h0 + HPC] = np.asarray(res.results[c]["outT"]).transpose(0, 2, 1)
    if _trace:
        return out, res
    return out
